# revision 3
# baseline (speedup 1.0000x reference)
"""XCA-style attention block (qkv 1x1 conv -> depthwise 3x3 -> L2-normed
cross-covariance attention -> 1x1 proj) on 8 TRN2 NeuronCores.

Sharding: core i handles (batch b = i//2, image half hf = i%2): 128 rows of
the 256-row image. The L2 norms and per-head [24,24] Gram matrices are
reductions over the full image, so each pair of cores all-reduces a tiny
[128,195] stats block; everything else is local. Softmax + temperature +
norm fixups fold into a single [192,192] matrix W2 = proj_w @
blockdiag(attn), so phase 2 is one matmul over v.

Key design points (v8, ~873us HW exec vs ~1001us for the first version):
- qkv lives in a row-padded SBUF array qB (row stride 258, data at +1,
  zero pads at row edges) written directly by the PSUM evacuation on
  ScalarE: horizontal conv taps read strided [P, rows, 256] views with no
  wrap errors, no shifted copy, no patch ops.
- qkv is computed on a 2-row sliding window (prev chunk tail DMA-copied),
  eliminating halo recompute; a 2-row tail step finishes the image.
- The q,k channels of the qkv matmul run as ONE fp8 DoubleRow matmul
  (192-contraction split 2x96) instead of two bf16 matmuls - the PE cost
  is output-rows * max(1, K/128) cycles, so DR halves it. fp8 error on
  q,k washes out in the px-averaged gram/ssq (logits are O(0.1),
  near-uniform softmax); v stays bf16 since its error reaches the output.
- dw conv taps: 5 on TensorE as diagonal-weight bf16 matmuls accumulated
  in PSUM (diag matmuls are low-power and dodge the dense-matmul
  utilization throttle), 4 on DVE as tensor_scalar(4x)+tensor_tensor(2x)
  pairs with 4B-aligned reads; ScalarE merges PSUM to bf16 acc.
- Transposes+gram for chunk c-1 are interleaved into chunk c tap stream
  to keep the PE busy (p-state ramp needs continuous execution).
- Phase 2 orders matmuls for lhsT reuse (LDWEIGHTS serializes with the
  prior matmul drain), splits PSUM evacuation across ScalarE+DVE, and
  prefetches 6 chunks of v during the collective/epilogue.
- Epilogue avoids SBUF->SBUF DMA hops via per-column PE transposes +
  rank-1 broadcast matmuls; softmax skips max-subtraction (|logits|<=1).

Measured: HW exec ~873us, rel err ~5.9e-3 vs the f32 reference.
"""
import numpy as np
import ml_dtypes

import concourse.bass as bass
import concourse.tile as tile
from concourse import mybir
from concourse.bass_utils import run_bass_kernel_spmd
from concourse.masks import make_identity

# --- patch: this walrus build rejects >1 semaphore wait on a Drain ---------
import concourse.tile as _tile_mod
from concourse.vector_clock import ScopedClock as _SC, VectorClock as _VC


def _drain_and_barrier(self, tick_clock, wait_clock):
    gc = tick_clock.global_clock
    n = len(gc)
    nonzero = [i for i in range(n) if gc[i] > 0]
    for i in nonzero:
        vec = [gc[j] if j == i else 0 for j in range(n)]
        inst = self.nc.sync.drain()
        wait_clock.add_sem_waits(inst.ins, _SC({None: _VC(vec)}))
    if not nonzero:
        inst = self.nc.sync.drain()
        wait_clock.add_sem_waits(inst.ins, _SC({None: gc}))
    self.nc.all_engine_barrier()
    assert self.sems is not None
    popped = self.nc._tile_sem_poison_stack.pop()
    assert popped is self._sem_poison
    self.nc.clear_and_free_semaphores(list(self.sems.allocated().values()))
    self.nc.all_engine_barrier()


_tile_mod.TileContext._drain_and_barrier = _drain_and_barrier

# The same walrus limit applies to every engine instruction: at most ONE
# semaphore wait. Split extra waits onto preceding same-engine NoOps (engines
# execute in order, so earlier waits still gate the instruction). DMA copies
# use the descriptor path and tolerate multiple waits, so leave them alone.
_orig_commit_and_lower = _tile_mod.TileContext._commit_and_lower
_split_counter = [0]


def _commit_and_lower_split(self, inst, original_block, old_bb_map, bb_to_exit_bb):
    si = getattr(inst, "sync_info", None)
    if si is not None and len(si.on_wait) > 1 and inst.engine is not None:
        waits = list(si.on_wait)
        for w in waits[:-1]:
            _split_counter[0] += 1
            nop = mybir.InstNoOp(
                name=f"{inst.name}-wsplit{_split_counter[0]}",
                sync_info=mybir.SyncInfo(on_wait=[w], on_update=[]),
                bass_nofuse=True,
                engine=inst.engine,
            )
            self._commit_instruction(nop)
        inst.sync_info = mybir.SyncInfo(on_wait=[waits[-1]], on_update=list(si.on_update))
    return _orig_commit_and_lower(self, inst, original_block, old_bb_map, bb_to_exit_bb)


_tile_mod.TileContext._commit_and_lower = _commit_and_lower_split
# ---------------------------------------------------------------------------

F32 = mybir.dt.float32
BF16 = mybir.dt.bfloat16
AX = mybir.AxisListType
OP = mybir.AluOpType
ACTF = mybir.ActivationFunctionType

B, C, H, W = 4, 192, 256, 256
HEADS, HD = 8, 24
C3 = 3 * C  # 576
HALF = H // 2  # rows per core
CH = 8  # qkv rows computed per chunk
NCH = HALF // CH  # 16 qkv chunks (+ 2-row tail)
NCC = NCH + 1  # 17 conv chunks
WP = W + 2  # padded row stride
PX = CH * W  # 2048
CT = [128, 128, 128, 128, 64]  # channel tiles over C3
CTO = [0, 128, 256, 384, 512]
NB = 512  # px per psum block

# tap split (dy, dx), dx in {0,1,2} = left/center/right column:
# - PE taps: center column (odd read offsets, PE doesn't care) + 2 corners
# - DVE taps: dx in {0,2} so strided reads stay 4B-aligned
PE_TAPS = [(1, 1), (0, 1), (2, 1), (0, 0), (2, 2)]
DVE_TAPS = [(0, 2), (1, 0), (1, 2), (2, 0)]
FP8 = mybir.dt.float8e4


def build_nc():
    nc = bass.Bass()
    x_ext = nc.declare_dram_parameter("xin", [C, (HALF + 2) * W], BF16, isOutput=False)
    x8_ext = nc.declare_dram_parameter("xin8", [96, 2 * (HALF + 2) * W], FP8, isOutput=False)
    qkvw8_ext = nc.declare_dram_parameter("qkvw8", [96, 768], FP8, isOutput=False)
    qkvwt_ext = nc.declare_dram_parameter("qkvwt", [C, C3], BF16, isOutput=False)
    projt_ext = nc.declare_dram_parameter("projt", [C, C], BF16, isOutput=False)
    dw9_ext = nc.declare_dram_parameter("dw9", [C3, 9], F32, isOutput=False)
    dwd_ext = nc.declare_dram_parameter("dwdiag", [len(PE_TAPS) * 128, C3], BF16, isOutput=False)
    tempcol_ext = nc.declare_dram_parameter("tempcol", [128, 2], F32, isOutput=False)
    out_ext = nc.declare_dram_parameter("out", [C, HALF * W], BF16, isOutput=True)

    with tile.TileContext(nc) as tc:
        with tc.tile_pool(name="wpool", bufs=1) as wp, \
             tc.tile_pool(name="dram", bufs=1, space="DRAM") as dram:
            # ---- weights / constants
            qkvw0 = wp.tile([128, C3], BF16)
            qkvw1 = wp.tile([128, C3], BF16)
            nc.sync.dma_start(out=qkvw0[:], in_=qkvwt_ext[0:128, :])
            nc.sync.dma_start(out=qkvw1[0:64, :], in_=qkvwt_ext[128:192, :])
            nc.sync.dma_start(out=qkvw1[64:128, :], in_=qkvwt_ext[128:192, :])
            projt0 = wp.tile([128, C], BF16)
            projt1 = wp.tile([64, C], BF16)
            nc.sync.dma_start(out=projt0[:], in_=projt_ext[0:128, :])
            nc.sync.dma_start(out=projt1[:], in_=projt_ext[128:192, :])
            dw9 = [wp.tile([CT[ct], 9], F32, name=f"dw9_{ct}") for ct in range(5)]
            for ct in range(5):
                nc.sync.dma_start(out=dw9[ct][:], in_=dw9_ext[CTO[ct]:CTO[ct] + CT[ct], :])
            qkvw8 = wp.tile([96, 768], FP8)
            nc.sync.dma_start(out=qkvw8[:], in_=qkvw8_ext[:])
            tempcol = wp.tile([128, 2], F32)
            nc.sync.dma_start(out=tempcol[:], in_=tempcol_ext[:])
            dwd = [wp.tile([128, C3], BF16, name=f"dwd{s}") for s in range(len(PE_TAPS))]
            for s in range(len(PE_TAPS)):
                nc.sync.dma_start(out=dwd[s][:], in_=dwd_ext[128 * s:128 * (s + 1), :])
            ident = wp.tile([128, 128], BF16)
            make_identity(nc, ident[:])
            ident32 = wp.tile([128, 128], F32)
            make_identity(nc, ident32[:])
            ones32 = wp.tile([1, 128], F32)
            nc.vector.memset(ones32[:], 1.0)

            # persistent accumulators
            sq_part = [wp.tile([CT[ct], NCC], F32, name=f"sqp{ct}") for ct in range(3)]
            v_dram = dram.tile([C, HALF * W], BF16)
            stats = wp.tile([128, 195], F32)

            # ================= phase 1 =================
            with tc.tile_pool(name="p1", bufs=2) as p1, \
                 tc.tile_pool(name="ps1", bufs=2, space="PSUM") as ps1, \
                 tc.tile_pool(name="gps", bufs=1, space="PSUM") as gps:
                gAB = gps.tile([96, 192], F32, tag="gAB")
                gA = gAB[:, 0:96]
                gB = gAB[:, 96:192]

                # deferred stats closures from the previous conv chunk,
                # spliced into the current chunk's tap stream
                deferred = []

                def emit_qkv(c, qB, prev_qB):
                    """qkv matmuls for chunk c (qkv rows 8c-1..8c+7) written
                    into qB rows 2..10 (strided, padded). Copies prev chunk's
                    qB rows 8,9 -> rows 0,1 (zeros for chunk 0)."""
                    for ct in range(5):
                        if prev_qB is None:
                            nc.gpsimd.memset(qB[ct][:, 0:2 * WP], 0.0)
                        else:
                            nc.sync.dma_start(out=qB[ct][:, 0:2 * WP],
                                              in_=prev_qB[ct][:, 8 * WP:10 * WP])
                        qr = qB[ct][:, 2 * WP:10 * WP].rearrange("p (r w) -> p r w", w=WP)
                        nc.gpsimd.memset(qr[:, :, 0:1], 0.0)
                        nc.gpsimd.memset(qr[:, :, WP - 1:WP], 0.0)

                    xrow0 = 8 * c * W
                    x0 = p1.tile([128, PX], BF16, tag="x0", bufs=3)
                    x1 = p1.tile([128, PX], BF16, tag="x1", bufs=3)
                    x8 = p1.tile([96, 2 * PX], FP8, tag="x8", bufs=3)
                    nc.sync.dma_start(out=x0[:], in_=x_ext[0:128, xrow0:xrow0 + PX])
                    nc.sync.dma_start(out=x1[0:64, :], in_=x_ext[128:192, xrow0:xrow0 + PX])
                    nc.sync.dma_start(out=x1[64:128, :], in_=x_ext[128:192, xrow0:xrow0 + PX])
                    HW2 = (HALF + 2) * W
                    for s in range(2):
                        nc.sync.dma_start(
                            out=x8[:, s * PX:(s + 1) * PX],
                            in_=x8_ext[:, s * HW2 + xrow0:s * HW2 + xrow0 + PX])
                    x8v = x8[:].rearrange("p (two f) -> p two f", two=2)

                    for ct in range(5):
                        cts = slice(CTO[ct], CTO[ct] + CT[ct])
                        w0 = qkvw0[:, cts]
                        for nb0 in range(0, 4, 2):
                            pair = [nb0, nb0 + 1]
                            tiles = []
                            for j, nb in enumerate(pair):
                                ps = ps1.tile([128, NB], F32, tag="qkvps", name=f"qps{j}")
                                o = ps[0:CT[ct], :]
                                if ct < 3:
                                    # q,k: full 192-contraction in ONE fp8
                                    # DoubleRow matmul (2x96 split-halves)
                                    lw = qkvw8[:, ct * 256:(ct + 1) * 256]
                                    lw = lw.rearrange("p (two f) -> p two f", two=2)
                                    nc.tensor.matmul(
                                        o, lw, x8v[:, :, nb * NB:(nb + 1) * NB],
                                        start=True, stop=True,
                                        perf_mode=mybir.MatmulPerfMode.DoubleRow)
                                else:
                                    nc.tensor.matmul(o, w0, x0[:, nb * NB:(nb + 1) * NB],
                                                     start=True, stop=False)
                                tiles.append((nb, ps, o))
                            if ct >= 3:
                                for j, (nb, ps, o) in enumerate(tiles):
                                    r0 = 64 * j
                                    nc.tensor.matmul(
                                        o, qkvw1[r0:r0 + 64, cts],
                                        x1[r0:r0 + 64, nb * NB:(nb + 1) * NB],
                                        start=False, stop=True,
                                        tile_position=(r0, 0))
                            for nb, ps, o in tiles:
                                ov = qB[ct][:, (2 + 2 * nb) * WP:(4 + 2 * nb) * WP]
                                ov = ov.rearrange("p (r w) -> p r w", w=WP)[:, :, 1:257]
                                nc.scalar.copy(ov, o.rearrange("p (r w) -> p r w", w=W))

                def emit_qkv_tail(qB, prev_qB):
                    """tail: qkv rows 127,128 -> qB rows 2..4."""
                    for ct in range(5):
                        nc.sync.dma_start(out=qB[ct][:, 0:2 * WP],
                                          in_=prev_qB[ct][:, 8 * WP:10 * WP])
                        qr = qB[ct][:, 2 * WP:4 * WP].rearrange("p (r w) -> p r w", w=WP)
                        nc.gpsimd.memset(qr[:, :, 0:1], 0.0)
                        nc.gpsimd.memset(qr[:, :, WP - 1:WP], 0.0)
                    xrow0 = 128 * W
                    xt0 = p1.tile([128, 2 * W], BF16, tag="xt0", bufs=1)
                    xt1 = p1.tile([128, 2 * W], BF16, tag="xt1", bufs=1)
                    xt8 = p1.tile([96, 4 * W], FP8, tag="xt8", bufs=1)
                    nc.sync.dma_start(out=xt0[:], in_=x_ext[0:128, xrow0:xrow0 + 2 * W])
                    nc.sync.dma_start(out=xt1[0:64, :], in_=x_ext[128:192, xrow0:xrow0 + 2 * W])
                    nc.sync.dma_start(out=xt1[64:128, :], in_=x_ext[128:192, xrow0:xrow0 + 2 * W])
                    HW2 = (HALF + 2) * W
                    for s in range(2):
                        nc.sync.dma_start(
                            out=xt8[:, s * 2 * W:(s + 1) * 2 * W],
                            in_=x8_ext[:, s * HW2 + xrow0:s * HW2 + xrow0 + 2 * W])
                    xt8v = xt8[:].rearrange("p (two f) -> p two f", two=2)
                    for ct in range(5):
                        cts = slice(CTO[ct], CTO[ct] + CT[ct])
                        ps = ps1.tile([128, NB], F32, tag="qkvps")
                        o = ps[0:CT[ct], :]
                        if ct < 3:
                            lw = qkvw8[:, ct * 256:(ct + 1) * 256]
                            lw = lw.rearrange("p (two f) -> p two f", two=2)
                            nc.tensor.matmul(o, lw, xt8v[:], start=True, stop=True,
                                             perf_mode=mybir.MatmulPerfMode.DoubleRow)
                        else:
                            nc.tensor.matmul(o, qkvw0[:, cts], xt0[:], start=True, stop=False)
                            nc.tensor.matmul(o, qkvw1[0:64, cts], xt1[0:64, :],
                                             start=False, stop=True, tile_position=(0, 0))
                        ov = qB[ct][:, 2 * WP:4 * WP]
                        ov = ov.rearrange("p (r w) -> p r w", w=WP)[:, :, 1:257]
                        nc.scalar.copy(ov, o.rearrange("p (r w) -> p r w", w=W))

                def splice():
                    if deferred:
                        deferred.pop(0)()

                def emit_taps(c, qB, acc, nrows):
                    """dw conv taps for conv chunk c (nrows output rows).
                    PE taps tap-outer over nbk one-bank PSUM blocks; DVE taps
                    as ts(4x)+tt(2x) pairs. acc[ct] = [CT, nrows*W] bf16."""
                    nbk = (nrows * W + NB - 1) // NB  # 512-px (2-row) blocks
                    qBr = [qB[ct][:, 0:(nrows + 2) * WP].rearrange(
                        "p (r w) -> p r w", w=WP) for ct in range(5)]
                    for ct in range(5):
                        dwps = [ps1.tile([128, NB], F32, tag="dwps", bufs=4,
                                         name=f"dwp{nb}") for nb in range(nbk)]
                        for si, (dy, dx) in enumerate(PE_TAPS):
                            for nb in range(nbk):
                                o = dwps[nb][0:CT[ct], :].rearrange(
                                    "p (r w) -> p r w", w=W)
                                src = qBr[ct][:, 2 * nb + dy:2 * nb + dy + 2, dx:dx + W]
                                nc.tensor.matmul(
                                    o, dwd[si][0:CT[ct], CTO[ct]:CTO[ct] + CT[ct]],
                                    src, start=(si == 0), stop=(si == len(PE_TAPS) - 1))
                        for nb in range(nbk):
                            nc.scalar.copy(acc[ct][:, nb * NB:(nb + 1) * NB],
                                           dwps[nb][0:CT[ct], :])
                        splice()
                        # DVE side taps (4B-aligned strided reads)
                        for (dy, dx) in DVE_TAPS:
                            wcol = dw9[ct][:, 3 * dy + dx:3 * dy + dx + 1]
                            src = qBr[ct][:, dy:dy + nrows, dx:dx + W]
                            tmp = p1.tile([CT[ct], PX], BF16, tag="dwtmp", name="dwtmp")
                            nc.vector.tensor_scalar(
                                out=tmp[:, 0:nrows * W], in0=src, scalar1=wcol,
                                scalar2=None, op0=OP.mult)
                            nc.vector.tensor_tensor(
                                out=acc[ct][:, 0:nrows * W], in0=acc[ct][:, 0:nrows * W],
                                in1=tmp[:, 0:nrows * W], op=OP.add)
                        splice()

                def make_stats_closures(c, acc, px0, px1, first, last):
                    """stats for conv chunk c over valid px [px0, px1):
                    ssq (Act) + v spill (DMA), then transposes+gram (PE) in
                    2-block groups."""
                    cls = []

                    def c_ssq_spill():
                        for ct in range(3):
                            scr = p1.tile([CT[ct], PX], BF16, tag="sqscr")
                            nc.scalar.activation(
                                out=scr[:, 0:px1 - px0], in_=acc[ct][:, px0:px1],
                                func=ACTF.Square,
                                accum_out=sq_part[ct][:, c:c + 1])
                        orow0 = (8 * c - 2) * W + px0 if c > 0 else 0
                        nc.sync.dma_start(out=v_dram[0:128, orow0:orow0 + px1 - px0],
                                          in_=acc[3][:, px0:px1])
                        nc.sync.dma_start(out=v_dram[128:192, orow0:orow0 + px1 - px0],
                                          in_=acc[4][:, px0:px1])
                    cls.append(c_ssq_spill)

                    nblk = (px1 - px0) // 128
                    for g0 in range(0, nblk, 2):
                        gn = min(2, nblk - g0)

                        def c_transp(g0=g0, gn=gn):
                            tg = ps1.tile([128, 768], BF16, tag="tps", bufs=1)
                            qkt = p1.tile([128, 768], BF16, tag="qkt", bufs=3)
                            for j in range(gn):
                                pbs = slice(px0 + (g0 + j) * 128, px0 + (g0 + j + 1) * 128)
                                nc.tensor.transpose(tg[:, 384 * j:384 * j + 128],
                                                    acc[0][:, pbs], ident[:])
                                nc.tensor.transpose(tg[:, 384 * j + 128:384 * j + 256],
                                                    acc[1][:, pbs], ident[:])
                                nc.tensor.transpose(tg[:, 384 * j + 256:384 * j + 384],
                                                    acc[2][:, pbs], ident[:])
                            nc.scalar.copy(qkt[:, 0:384 * gn], tg[:, 0:384 * gn])
                            for j in range(gn):
                                qj = qkt[:, 384 * j:384 * (j + 1)]
                                st = first and g0 == 0 and j == 0
                                sp = last and g0 + j == nblk - 1
                                nc.tensor.matmul(gA, qj[:, 0:96], qj[:, 192:288],
                                                 start=st, stop=sp)
                                nc.tensor.matmul(gB, qj[:, 96:192], qj[:, 288:384],
                                                 start=st, stop=sp)
                        cls.append(c_transp)
                    return cls

                # ---- main loop over 17 conv chunks
                prev_qB = None
                for c in range(NCC):
                    nrows = 8 if c < NCH else 2
                    qB = [p1.tile([CT[ct], (nrows + 2) * WP], BF16,
                                  tag=f"qB{ct}", name=f"qB{ct}", bufs=2)
                          for ct in range(5)]
                    if c < NCH:
                        emit_qkv(c, qB, prev_qB)
                    else:
                        emit_qkv_tail(qB, prev_qB)
                    acc = [p1.tile([CT[ct], PX], BF16, tag=f"acc{ct}",
                                   name=f"acc{ct}", bufs=3) for ct in range(5)]
                    emit_taps(c, qB, acc, nrows)
                    px0 = 512 if c == 0 else 0
                    px1 = nrows * W
                    deferred.extend(make_stats_closures(
                        c, acc, px0, px1, first=(c == 0), last=(c == NCC - 1)))
                    prev_qB = qB
                while deferred:
                    deferred.pop(0)()

                # fold chunk partials; pack stats = [gA | gB | ssq]
                for ct in range(3):
                    nc.vector.tensor_reduce(
                        out=stats[0:CT[ct], 192 + ct:193 + ct],
                        in_=sq_part[ct][:], axis=AX.X, op=OP.add)
                nc.scalar.copy(stats[0:96, 0:192], gAB[:])

            # ================= collective =================
            cc_in = dram.tile([128, 195], F32)
            cc_out = dram.tile([128, 195], F32)
            nc.sync.dma_start(out=cc_in[:], in_=stats[:])
            nc.gpsimd.collective_compute(
                "AllReduce", OP.add,
                replica_groups=[[0, 1], [2, 3], [4, 5], [6, 7]],
                ins=[cc_in.opt()], outs=[cc_out.opt()])
            statf = wp.tile([128, 195], F32)
            nc.sync.dma_start(out=statf[:], in_=cc_out[:])

            # ================= epilogue (tiny) =================
            with tc.tile_pool(name="ep", bufs=1) as ep:
              with tc.tile_pool(name="eps", bufs=1, space="PSUM") as eps:
                # 1/max(sqrt(ssq), eps) per q/k channel, [128, 3] by ct
                nrm = ep.tile([128, 3], F32)
                nc.scalar.activation(out=nrm[:], in_=statf[:, 192:195], func=ACTF.Sqrt)
                nc.vector.tensor_scalar(out=nrm[:], in0=nrm[:], scalar1=1e-12,
                                        scalar2=None, op0=OP.max)
                rn = ep.tile([128, 3], F32)
                nc.vector.reciprocal(rn[:], nrm[:])

                # row scales (q-norms * temperature), partition-packed per gram tile
                rsA = ep.tile([96, 1], F32)
                nc.vector.tensor_tensor(out=rsA[:], in0=rn[0:96, 0:1],
                                        in1=tempcol[0:96, 0:1], op=OP.mult)
                # partition-offset rearrangements go through SBUF->SBUF DMA:
                # DVE writes at non-quadrant-aligned partition bases are illegal
                rsB = ep.tile([96, 1], F32)
                nc.sync.dma_start(out=rsB[0:32, :], in_=rn[96:128, 0:1])
                nc.sync.dma_start(out=rsB[32:96, :], in_=rn[0:64, 1:2])
                nc.vector.tensor_tensor(out=rsB[:], in0=rsB[:],
                                        in1=tempcol[0:96, 1:2], op=OP.mult)

                # column scales (k-norms): per-column transposes land k-norms
                # on partition 0 as free-dim rows, then rank-1 matmuls build
                # the [96, 96] broadcasts (no SBUF->SBUF DMA hops)
                tps = eps.tile([1, 256], F32, tag="t")
                nc.tensor.transpose(tps[:, 0:128], rn[:, 1:2], ident32[:])
                nc.tensor.transpose(tps[:, 128:256], rn[:, 2:3], ident32[:])
                rnT = ep.tile([1, 256], F32)
                nc.vector.tensor_copy(rnT[:], tps[:])
                bcps = eps.tile([96, 96], F32, tag="bc")
                bc = [ep.tile([96, 96], F32, name=f"bc{g}") for g in range(2)]
                # bc0 cols = k-norm ch 0..96 = rn[64:128, 1] | rn[0:32, 2]
                nc.tensor.matmul(bcps[:, 0:64], ones32[0:1, 0:96],
                                 rnT[0:1, 64:128], start=True, stop=True)
                nc.tensor.matmul(bcps[:, 64:96], ones32[0:1, 0:96],
                                 rnT[0:1, 128:160], start=True, stop=True)
                nc.vector.tensor_copy(bc[0][:], bcps[:])
                # bc1 cols = k-norm ch 96..192 = rn[32:128, 2]
                nc.tensor.matmul(bcps[:], ones32[0:1, 0:96],
                                 rnT[0:1, 160:256], start=True, stop=True)
                nc.vector.tensor_copy(bc[1][:], bcps[:])

                # logits = gram * rq * rk * temp; diag-extract -> [96, 24] per tile
                attn = []
                for g in range(2):
                    lg = ep.tile([96, 96], F32, name=f"lg{g}")
                    nc.vector.tensor_scalar(out=lg[:], in0=statf[0:96, 96 * g:96 * (g + 1)],
                                            scalar1=(rsA if g == 0 else rsB)[:],
                                            scalar2=None, op0=OP.mult)
                    nc.vector.tensor_tensor(out=lg[:], in0=lg[:], in1=bc[g][:], op=OP.mult)
                    sm = ep.tile([96, HD], F32, name=f"sm{g}")
                    for hl in range(4):
                        nc.sync.dma_start(out=sm[24 * hl:24 * (hl + 1), :],
                                          in_=lg[24 * hl:24 * (hl + 1), 24 * hl:24 * (hl + 1)])
                    # logits = q-hat . k-hat * temp are bounded (|.| <= temp):
                    # exp is safe without the max-subtraction pass
                    ex = ep.tile([96, HD], F32, name=f"ex{g}")
                    nc.scalar.activation(out=ex[:], in_=sm[:], func=ACTF.Exp)
                    sme = ep.tile([96, 1], F32, name=f"sme{g}")
                    nc.vector.tensor_reduce(out=sme[:], in_=ex[:], axis=AX.X, op=OP.add)
                    rs = ep.tile([96, 1], F32, name=f"rs{g}")
                    nc.vector.reciprocal(rs[:], sme[:])
                    at = ep.tile([96, HD], BF16, name=f"at{g}")
                    nc.vector.tensor_scalar(out=at[:], in0=ex[:], scalar1=rs[:],
                                            scalar2=None, op0=OP.mult)
                    attn.append(at)

                # blockdiag(attn) as lhsT rows=out-chan(24h+d), cols=v-chan(24h+e)
                abd0 = ep.tile([128, C], BF16)
                abd1 = ep.tile([64, C], BF16)
                nc.vector.memset(abd0[:], 0.0)
                nc.vector.memset(abd1[:], 0.0)
                for h in range(HEADS):
                    g, hl = divmod(h, 4)
                    src = attn[g]
                    r0, cc0 = 24 * h, 24 * h
                    if r0 + 24 <= 128:
                        nc.sync.dma_start(out=abd0[r0:r0 + 24, cc0:cc0 + 24],
                                          in_=src[24 * hl:24 * hl + 24, :])
                    elif r0 >= 128:
                        nc.sync.dma_start(out=abd1[r0 - 128:r0 - 104, cc0:cc0 + 24],
                                          in_=src[24 * hl:24 * hl + 24, :])
                    else:
                        k0 = 128 - r0
                        nc.sync.dma_start(out=abd0[r0:128, cc0:cc0 + 24],
                                          in_=src[24 * hl:24 * hl + k0, :])
                        nc.sync.dma_start(out=abd1[0:24 - k0, cc0:cc0 + 24],
                                          in_=src[24 * hl + k0:24 * hl + 24, :])

                # W2T[c, o] = sum_r abd[r, c] * projt[r, o]
                w2t0 = ep.tile([128, C], BF16)
                w2t1 = ep.tile([64, C], BF16)
                wps = eps.tile([128, C], F32, tag="wps")
                nc.tensor.matmul(wps[:], abd0[:, 0:128], projt0[:], start=True, stop=False)
                nc.tensor.matmul(wps[:], abd1[:, 0:128], projt1[:], start=False, stop=True)
                nc.scalar.copy(w2t0[:], wps[:])
                wps2 = eps.tile([64, C], F32, tag="wps2")
                nc.tensor.matmul(wps2[:], abd0[:, 128:192], projt0[:], start=True, stop=False)
                nc.tensor.matmul(wps2[:], abd1[:, 128:192], projt1[:], start=False, stop=True)
                nc.scalar.copy(w2t1[:], wps2[:])

              # ================= phase 2: out = W2 @ v =================
              # DVE does the PSUM evacuation (idle in phase 2; ScalarE did
              # the epilogue), out-tile copies batched per chunk.
              with tc.tile_pool(name="p2", bufs=2) as p2, \
                   tc.tile_pool(name="ps2", bufs=2, space="PSUM") as ps2:
                  vbs = {}
                  for c in range(6):  # prefetch first 6 chunks' v
                      vb0 = p2.tile([128, PX], BF16, tag="vb0", bufs=6)
                      vb1 = p2.tile([64, PX], BF16, tag="vb1", bufs=6)
                      vbs[c] = (vb0, vb1)
                      cs = slice(c * PX, (c + 1) * PX)
                      nc.sync.dma_start(out=vb0[:], in_=v_dram[0:128, cs])
                      nc.sync.dma_start(out=vb1[:], in_=v_dram[128:192, cs])
                  for c in range(NCH):
                      cs = slice(c * PX, (c + 1) * PX)
                      if c < 6:
                          vb0, vb1 = vbs[c]
                      else:
                          vb0 = p2.tile([128, PX], BF16, tag="vb0", bufs=6)
                          vb1 = p2.tile([64, PX], BF16, tag="vb1", bufs=6)
                          nc.sync.dma_start(out=vb0[:], in_=v_dram[0:128, cs])
                          nc.sync.dma_start(out=vb1[:], in_=v_dram[128:192, cs])
                      ob0 = p2.tile([128, PX], BF16, tag="ob0", bufs=2)
                      ob1 = p2.tile([64, PX], BF16, tag="ob1", bufs=2)
                      # lhsT-reuse order: LDWEIGHTS serializes with the prior
                      # matmul's drain, so keep each stationary loaded across
                      # all 4 px blocks (4 LDW/chunk instead of 16)
                      f0 = ps2.tile([128, PX], F32, tag="f0", bufs=1)
                      f1 = ps2.tile([64, PX], F32, tag="f1", bufs=1)
                      for nb in range(4):
                          nc.tensor.matmul(f0[:, nb * NB:(nb + 1) * NB],
                                           w2t0[:, 0:128], vb0[:, nb * NB:(nb + 1) * NB],
                                           start=True, stop=False)
                      for nb in range(4):
                          nc.tensor.matmul(f0[:, nb * NB:(nb + 1) * NB],
                                           w2t1[:, 0:128], vb1[:, nb * NB:(nb + 1) * NB],
                                           start=False, stop=True)
                      for nb in range(4):
                          nc.tensor.matmul(f1[:, nb * NB:(nb + 1) * NB],
                                           w2t0[:, 128:192], vb0[:, nb * NB:(nb + 1) * NB],
                                           start=True, stop=False)
                      for nb in range(4):
                          nc.tensor.matmul(f1[:, nb * NB:(nb + 1) * NB],
                                           w2t1[:, 128:192], vb1[:, nb * NB:(nb + 1) * NB],
                                           start=False, stop=True)
                      # split evacuation across Act and DVE
                      # per-half evac (Act takes ob0, DVE ob1 in parallel)
                      # with the half-0 store streaming out while half 1
                      # still evacuates
                      for hf in range(2):
                          hs = slice(hf * 1024, hf * 1024 + 1024)
                          ds = slice(c * PX + hf * 1024, c * PX + hf * 1024 + 1024)
                          nc.scalar.copy(ob0[:, hs], f0[:, hs])
                          nc.vector.tensor_copy(ob1[:, hs], f1[:, hs])
                          nc.sync.dma_start(out=out_ext[0:128, ds], in_=ob0[:, hs])
                          nc.sync.dma_start(out=out_ext[128:192, ds], in_=ob1[:, hs])
    return nc


_NC_CACHE = None


def _get_nc():
    global _NC_CACHE
    if _NC_CACHE is None:
        _NC_CACHE = build_nc()
    return _NC_CACHE


def _shard_inputs(x, qkv_w, dw_w, proj_w, temperature):
    qkvwt = np.ascontiguousarray(qkv_w.T).astype(ml_dtypes.bfloat16)
    # fp8 split-half lhsT for the qk qkv DoubleRow matmuls:
    # qkvw8[k, ct*256 + s*128 + m] = qkv_w[CTO[ct]+m, 96s+k]
    qkvw8 = np.zeros((96, 768), np.float32)
    for ct in range(3):
        for s in range(2):
            qkvw8[:, ct * 256 + s * 128:ct * 256 + s * 128 + 128] = \
                qkv_w[128 * ct:128 * ct + 128, 96 * s:96 * s + 96].T
    qkvw8 = qkvw8.astype(ml_dtypes.float8_e4m3)
    projt = np.ascontiguousarray(proj_w.T).astype(ml_dtypes.bfloat16)
    dw9 = np.ascontiguousarray(dw_w.reshape(C3, 9)).astype(np.float32)
    # per-PE-tap diagonal weight blocks: dwdiag[s, i, CTO[ct]+i] = w(tap_s, ch)
    dwdiag = np.zeros((len(PE_TAPS), 128, C3), np.float32)
    for s, (dy, dx) in enumerate(PE_TAPS):
        wv = dw9[:, 3 * dy + dx]
        for ct in range(5):
            idx = np.arange(CT[ct])
            dwdiag[s, idx, CTO[ct] + idx] = wv[CTO[ct] + idx]
    dwdiag = dwdiag.reshape(len(PE_TAPS) * 128, C3).astype(ml_dtypes.bfloat16)
    temp = np.asarray(temperature).reshape(HEADS)
    tempcol = np.zeros((128, 2), np.float32)
    for h in range(HEADS):
        g, hl = divmod(h, 4)
        tempcol[24 * hl:24 * (hl + 1), g] = temp[h]

    in_maps = []
    for i in range(8):
        b, hf = divmod(i, 2)
        xin = np.zeros((C, HALF + 2, W), np.float32)
        r0 = hf * HALF - 1
        lo, hi = max(r0, 0), min(r0 + HALF + 2, H)
        xin[:, lo - r0:hi - r0, :] = x[b, :, lo:hi, :]
        xf = xin.reshape(C, (HALF + 2) * W)
        xin8 = np.stack([xf[0:96], xf[96:192]], axis=1)  # [96, 2, HW2]
        in_maps.append({
            "xin": xf.astype(ml_dtypes.bfloat16),
            "xin8": xin8.reshape(96, -1).astype(ml_dtypes.float8_e4m3),
            "qkvw8": qkvw8, "qkvwt": qkvwt, "projt": projt, "dw9": dw9,
            "tempcol": tempcol, "dwdiag": dwdiag,
        })
    return in_maps


def kernel(x, qkv_w, dw_w, proj_w, temperature):
    nc = _get_nc()
    in_maps = _shard_inputs(x, qkv_w, dw_w, proj_w, temperature)
    res = run_bass_kernel_spmd(nc, in_maps, core_ids=list(range(8)))
    out = np.empty((B, C, H, W), np.float32)
    for i in range(8):
        b, hf = divmod(i, 2)
        o = res.results[i]["out"].astype(np.float32).reshape(C, HALF, W)
        out[b, :, hf * HALF:(hf + 1) * HALF, :] = o
    return out
